# revision 18
# baseline (speedup 1.0000x reference)
"""Trainium2 Bass kernel for nn_Chan_spaAtt (SE-gated conv block), v3.

The spatial self-attention branch in the reference is dead code -- the output
depends only on xo = x * sigmoid(xl + xg) through the final 3x3 conv + BN +
ReLU (all BN affines folded host-side):

  t1   = relu(W1 @ x + b1)                      [16, N]
  d    = G2 @ relu(G1 @ mean(x) + bg1) + bsig   [64, 1]
  sarg = W2 @ t1                                [64, N]
  xo   = x * sigmoid(sarg + d)                  [64, N]
  y    = relu(conv3x3(xo, CW) + cb)             [64, N]

Sharding: one sample per NeuronCore (B=8).

Layout: everything bf16 on-chip, 128 partitions everywhere.
 - x_dual [128, 4096]: partition c+64s holds x[c, row+s] per 8-row chunk.
   The SE phase computes each pixel twice (once per shift) at zero extra
   cost: engine time scales with the free dim only.
 - xo_pad [128, 40*132]: copy A (partitions 0:64) = padded xo grid with
   row stride 66; copy B (64:128) holds the next row's values at the same
   column (written directly by the dual-layout SE multiply).
 - conv3x3 = 6 dense K=128 matmuls per 8-row half-tile: M=128 packs (out
   channel x output-row-parity), K=128 packs (in channel x row shift).
   12288 PE rows total vs 24576 in the 9-tap formulation.  The first
   half-tile replaces its three K=128 taps with K=64 pairs so it never
   reads copy B's unwritten row -1 column (no fixup DMA on the chain).
 - global-branch mean via DVE reduce over a [128, 2048] half-stacked copy
   of x; a stacked-G1 f32r matmul recombines the partition halves exactly.
 - DMA: HWDGE descriptor-gen costs a flat ~625ns serialized per transfer,
   so x_dual rides the Pool-engine SWDGE path and everything else is
   batched into few transfers, ordered so the mean-reduce stream lands
   first.
"""

import sys

if "/opt/trn_rl_repo" not in sys.path:
    sys.path.insert(0, "/opt/trn_rl_repo")

import numpy as np
import ml_dtypes

import concourse.bass as bass
import concourse.bacc as bacc
import concourse.mybir as mybir
import concourse.tile as tile
from concourse.bass_utils import run_bass_kernel_spmd

B, C, H, W = 8, 64, 64, 64
N = H * W
C2 = 2 * C          # 128
INTER = 16
EPS = 1e-5
PW = W + 2          # padded row stride = 66
BW = 2 * PW         # conv-view block width = 132 (one row pair)
NBLK = 40           # blocks in xo_pad; 40*132 = 5280 columns
PADC = NBLK * BW
HEAD = PW + 1       # flat offset of grid pixel (0, 0) = 67
CHUNK = 512
NCHUNK = N // CHUNK          # 8
HALF = 256                   # conv half-tile free size (4 row pairs)

TAPS = ((-1, -1), (-1, 0), (-1, 1), (1, -1), (1, 0), (1, 1))

F32 = mybir.dt.float32
F32R = mybir.dt.float32r
BF16 = mybir.dt.bfloat16
AF = mybir.ActivationFunctionType
ALU = mybir.AluOpType
BFNP = ml_dtypes.bfloat16

# weight blob (bf16, 128 partitions) column layout
O_W1P = 0      # [128, 32]
O_W2P = 32     # [32, 128] on partitions 0:32
O_GW1F = 160   # [128, 64] = [128, 32] f32 (bitcast), stacked G1
O_GW2P = 224   # [16, 128] on partitions 0:16
O_BIAS = 352   # 4 f32-as-2xbf16? no: 4 bf16 cols: b1 | bsig | cb | gb1
WCOLS = 356
# cw6 blob: 6 dense taps [128, 768] + cw3b (s=1 rows of the d=-1 taps,
# re-homed to partitions 0:64) [64, 384] at cols 768:1152
CW_COLS = 1152

XH_SPLITS = ((0, 768), (768, 1536), (1536, 2048))
XD_SPLITS = ((0, 1024), (1024, 2048), (2048, 3072), (3072, 4096))

_prog_cache = {}


def _pix(r, w):
    """Flat column of valid grid pixel (r, w) in xo_pad copy A."""
    return HEAD + r * PW + w


def build_program(n_cores=8):
    nc = bacc.Bacc("TRN2", debug=False, target_bir_lowering=False,
                   num_devices=n_cores)

    wsm_d = nc.dram_tensor("wsm", [C2, WCOLS], BF16, kind="ExternalInput").ap()
    xh_d = nc.dram_tensor("xh", [C2, N // 2], BF16, kind="ExternalInput").ap()
    xd_d = nc.dram_tensor("xd", [C2, N], BF16, kind="ExternalInput").ap()
    cw6_d = nc.dram_tensor("cw6", [C2, CW_COLS], BF16,
                           kind="ExternalInput").ap()
    y_d = nc.dram_tensor("y", [C2, N // 2], BF16, kind="ExternalOutput").ap()

    with tile.TileContext(nc) as tc:
        with tc.tile_pool(name="big", bufs=1) as bpool, \
             tc.tile_pool(name="work", bufs=3) as wpool, \
             tc.tile_pool(name="t1s", bufs=8) as tpool, \
             tc.tile_pool(name="ps1p", bufs=2, space="PSUM") as pp1, \
             tc.tile_pool(name="ps2p", bufs=3, space="PSUM") as pp2, \
             tc.tile_pool(name="psyp", bufs=3, space="PSUM") as ppy:

            # dummy sigmoid at t~0: forces the single needed ACT table set
            # (sigmoid_and_others: sigmoid + relu + identity) to load early.
            scr = bpool.tile([1, 1], F32, tag="scr")
            nc.vector.memset(scr[:], 0)
            nc.scalar.activation(scr[:], scr[:], AF.Sigmoid)

            # ---- input DMAs.  SP/HWDGE: weights, chunk 0-1, then the
            # mean-reduce stream.  Pool/SWDGE: remaining x_dual pieces. ----
            wsm = bpool.tile([C2, WCOLS], BF16, tag="wsm")
            nc.sync.dma_start(wsm[:], wsm_d)
            xd = bpool.tile([C2, N], BF16, tag="xd")
            lo, hi = XD_SPLITS[0]
            nc.sync.dma_start(xd[:, lo:hi], xd_d[:, lo:hi])
            xh = bpool.tile([C2, N // 2], BF16, tag="xh")
            for lo, hi in XH_SPLITS:
                nc.sync.dma_start(xh[:, lo:hi], xh_d[:, lo:hi])
            cw6 = bpool.tile([C2, CW_COLS], BF16, tag="cw6")
            nc.sync.dma_start(cw6[:], cw6_d)

            # ---- xo_pad halo memsets, split over Pool and DVE dead time ----
            xo_pad = bpool.tile([C2, PADC], BF16, tag="xopad")
            nc.gpsimd.memset(xo_pad[:, 0:HEAD], 0)
            # copy B's slot for grid row 64 (the bottom halo) stays zero
            nc.gpsimd.memset(xo_pad[C:C2, _pix(H - 1, 0):_pix(H - 1, W)], 0)
            gaps = xo_pad[:, HEAD + W:HEAD + W + H * PW]
            gaps = gaps.rearrange("p (r w) -> p r w", w=PW)[:, :, 0:2]
            nc.vector.memset(gaps, 0)
            # cover reads up to conv block 32 (col 4356); beyond is never read
            nc.vector.memset(xo_pad[:, _pix(H - 1, W) + 2:33 * BW], 0)

            for lo, hi in XD_SPLITS[1:]:
                nc.gpsimd.dma_start(xd[:, lo:hi], xd_d[:, lo:hi])

            w1p = wsm[:, O_W1P:O_W1P + 32]
            w2p = wsm[0:32, O_W2P:O_W2P + C2]
            gw1f = wsm[:, O_GW1F:O_GW1F + 32].bitcast(F32R)
            gw2p = wsm[0:INTER, O_GW2P:O_GW2P + C2]

            # ---- global mean partials: two DVE reduces + one ACT
            # identity-accumulate over the half-stacked copy ----
            gparts = wpool.tile([C2, 2], F32, tag="gparts")
            for q in range(2):
                lo, hi = XH_SPLITS[q]
                nc.vector.reduce_sum(gparts[:, q:q + 1], xh[:, lo:hi],
                                     axis=mybir.AxisListType.X)
            gacc = wpool.tile([C2, 1], F32, tag="gacc")
            trash = wpool.tile([C2, XH_SPLITS[2][1] - XH_SPLITS[2][0]], BF16,
                               tag="trash")
            nc.scalar.activation(trash[:],
                                 xh[:, XH_SPLITS[2][0]:XH_SPLITS[2][1]],
                                 AF.Identity, accum_out=gacc[:])

            # ---- f32 bias columns, converted on-chip from the bf16 blob ----
            fbias = wpool.tile([C2, 4], F32, tag="fbias")
            nc.vector.tensor_copy(fbias[:], wsm[:, O_BIAS:O_BIAS + 4])
            b1 = fbias[0:32, 0:1]
            bsig = fbias[:, 1:2]
            cb = fbias[:, 2:3]
            gb1 = fbias[0:INTER, 3:4]

            # ---- SE phase 1: mm1 for every chunk; t1 relu spread over
            # ACT (c0), Pool (odd), DVE (even, emitted inside phase 2) ----
            t1s = {}
            ps1s = {}

            def emit_mm1(ci):
                ps1 = pp1.tile([32, CHUNK], F32, tag="ps1")
                nc.tensor.matmul(ps1[:], w1p,
                                 xd[:, ci * CHUNK:(ci + 1) * CHUNK],
                                 start=True, stop=True)
                ps1s[ci] = ps1

            def emit_t1(ci, eng):
                t1 = tpool.tile([32, CHUNK], BF16, tag="t1")
                if eng == "act":
                    nc.scalar.activation(t1[:], ps1s[ci][:], AF.Relu, bias=b1)
                elif eng == "pool":
                    nc.gpsimd.tensor_scalar(t1[:], ps1s[ci][:], b1, 0.0,
                                            ALU.add, ALU.max)
                else:
                    nc.vector.tensor_scalar(t1[:], ps1s[ci][:], b1, 0.0,
                                            ALU.add, ALU.max)
                t1s[ci] = t1

            emit_mm1(0)
            emit_t1(0, "pool")
            emit_mm1(1)
            emit_t1(1, "pool")

            # ---- global branch MLP; gmm1 accumulates the three mean
            # partials directly (no combine reduce) ----
            psg1 = ppy.tile([INTER, 1], F32, tag="psy")
            nc.tensor.matmul(psg1[:], gw1f, gparts[:, 0:1].bitcast(F32R),
                             start=True, stop=False)
            nc.tensor.matmul(psg1[:], gw1f, gparts[:, 1:2].bitcast(F32R),
                             start=False, stop=False)
            nc.tensor.matmul(psg1[:], gw1f, gacc[:].bitcast(F32R),
                             start=False, stop=True)
            g1 = wpool.tile([INTER, 1], BF16, tag="g1")
            nc.scalar.activation(g1[:], psg1[:], AF.Relu, bias=gb1,
                                 scale=1.0 / N)
            psg2 = pp2.tile([C2, 1], F32, tag="ps2")
            nc.tensor.matmul(psg2[:], gw2p, g1[:], start=True, stop=True)
            dbias = wpool.tile([C2, 1], F32, tag="dbias")
            nc.scalar.activation(dbias[:], psg2[:], AF.Identity, bias=bsig)

            # ---- SE phase 2 + conv, software-pipelined ----
            def emit_mm2_sig(ci):
                ps2 = pp2.tile([C2, CHUNK], F32, tag="ps2")
                nc.tensor.matmul(ps2[:], w2p, t1s[ci][:],
                                 start=True, stop=True)
                sig = wpool.tile([C2, CHUNK], BF16, tag="sig")
                nc.scalar.activation(sig[:], ps2[:], AF.Sigmoid,
                                     bias=dbias[:])
                return sig

            def mul_rows(ci, sig, r0, nrow, top_only=False):
                pbase = C if top_only else C2
                off = (r0 - 8 * ci) * W
                dst = xo_pad[0:pbase, _pix(r0, 0):_pix(r0, 0) + nrow * PW]
                dst = dst.rearrange("p (r w) -> p r w", w=PW)[:, :, 0:W]
                xcr = xd[0:pbase, ci * CHUNK + off:ci * CHUNK + off + nrow * W]
                xcr = xcr.rearrange("p (r w) -> p r w", w=W)
                sgr = sig[0:pbase, off:off + nrow * W]
                sgr = sgr.rearrange("p (r w) -> p r w", w=W)
                nc.vector.tensor_mul(dst, xcr, sgr)

            def emit_mul(ci, sig):
                if ci < NCHUNK - 1:
                    mul_rows(ci, sig, 8 * ci, 8)
                else:
                    # split so conv h6 (needs only row 56) unblocks early;
                    # the bottom half's value for row 64 is never written
                    # (copy B's slot for it is the zero bottom halo)
                    mul_rows(ci, sig, 56, 1)
                    mul_rows(ci, sig, 57, 6)
                    mul_rows(ci, sig, 63, 1, top_only=True)

            # ---- conv3x3: 6 dense 128x128 matmuls per 8-row half-tile;
            # half 0 splits its d=-1 taps into K=64 pairs (no copy-B read
            # of the unwritten row -1 column). ----
            xor_v = xo_pad[:].rearrange("p (t w) -> p t w", w=BW)
            # shifted top-half view whose row-pair t holds content row 2t
            # (used by half 0 in place of copy B)
            xob_v = xo_pad[0:C, PW:PW + 4 * BW]
            xob_v = xob_v.rearrange("p (t w) -> p t w", w=BW)
            ysb = bpool.tile([C2, N // 2], BF16, tag="ysb")

            def emit_conv_half(j):
                psy = ppy.tile([C2, HALF], F32, tag="psy")
                first = True
                for dlt, dx in ((1, -1), (1, 0), (1, 1),
                                (-1, -1), (-1, 0), (-1, 1)):
                    jj = TAPS.index((dlt, dx))
                    wcol = jj * C2
                    t0 = (8 * j + dlt + 1) // 2
                    if j == 0 and dlt == -1:
                        nc.tensor.matmul(
                            psy[:], cw6[0:C, wcol:wcol + C2],
                            xor_v[0:C, t0:t0 + 4, 1 + dx:1 + dx + W],
                            start=False, stop=False)
                        kk = 768 + jj * C2
                        nc.tensor.matmul(
                            psy[:], cw6[0:C, kk:kk + C2],
                            xob_v[:, 0:4, 1 + dx:1 + dx + W],
                            start=False, stop=(dx == 1))
                    else:
                        nc.tensor.matmul(
                            psy[:], cw6[:, wcol:wcol + C2],
                            xor_v[:, t0:t0 + 4, 1 + dx:1 + dx + W],
                            start=first, stop=(j > 0 and dlt == -1
                                               and dx == 1))
                    first = False
                dsty = ysb[:, j * HALF:(j + 1) * HALF]
                if j < 4:
                    nc.gpsimd.tensor_scalar(dsty, psy[:], cb, 0.0,
                                            ALU.add, ALU.max)
                else:
                    nc.scalar.activation(dsty, psy[:], AF.Relu, bias=cb)

            sig0 = emit_mm2_sig(0)
            emit_mul(0, sig0)
            emit_mm1(2)
            emit_t1(2, "dve")
            sig1 = emit_mm2_sig(1)
            emit_mul(1, sig1)
            emit_mm1(3)
            emit_t1(3, "pool")
            sig2 = emit_mm2_sig(2)
            emit_mul(2, sig2)
            emit_mm1(4)
            emit_t1(4, "dve")
            emit_conv_half(0)
            sig3 = emit_mm2_sig(3)
            emit_mul(3, sig3)
            emit_mm1(5)
            emit_t1(5, "pool")
            emit_conv_half(1)
            sig4 = emit_mm2_sig(4)
            emit_mul(4, sig4)
            emit_mm1(6)
            emit_t1(6, "dve")
            emit_conv_half(2)
            sig5 = emit_mm2_sig(5)
            emit_mul(5, sig5)
            emit_mm1(7)
            emit_t1(7, "pool")
            emit_conv_half(3)
            sig6 = emit_mm2_sig(6)
            emit_mul(6, sig6)
            emit_conv_half(4)
            sig7 = emit_mm2_sig(7)
            emit_mul(7, sig7)
            emit_conv_half(5)
            nc.sync.dma_start(y_d[:, 0:4 * HALF], ysb[:, 0:4 * HALF])
            emit_conv_half(6)
            emit_conv_half(7)
            nc.sync.dma_start(y_d[:, 4 * HALF:7 * HALF],
                              ysb[:, 4 * HALF:7 * HALF])
            nc.sync.dma_start(y_d[:, 7 * HALF:8 * HALF],
                              ysb[:, 7 * HALF:8 * HALF])

    nc.compile()
    return nc


def _affine(s, b, m, v):
    inv = s / np.sqrt(v + EPS)
    return inv, b - m * inv


def prepare_weights(inputs):
    f = lambda k: np.asarray(inputs[k], dtype=np.float32)
    a1, c1 = _affine(f("ls1"), f("lbb1"), f("lm1"), f("lv1"))
    W1 = a1[:, None] * f("lw1")              # [16, 64]
    B1 = a1 * f("lb1") + c1
    a2, c2 = _affine(f("ls2"), f("lbb2"), f("lm2"), f("lv2"))
    W2 = a2[:, None] * f("lw2")              # [64, 16]
    B2 = a2 * f("lb2") + c2
    ag1, cg1 = _affine(f("gs1"), f("gbb1"), f("gm1"), f("gv1"))
    G1 = ag1[:, None] * f("gw1")             # [16, 64]
    Bg1 = ag1 * f("gb1") + cg1
    ag2, cg2 = _affine(f("gs2"), f("gbb2"), f("gm2"), f("gv2"))
    G2 = ag2[:, None] * f("gw2")             # [64, 16]
    Bg2 = ag2 * f("gb2") + cg2
    ac, cc = _affine(f("cs"), f("cbb"), f("cm"), f("cv"))
    CW = ac[:, None, None, None] * f("cw")   # [64, 64, 3, 3] (o, c, ky, kx)
    CB = ac * f("cb") + cc

    w1p = np.zeros((C2, 32), np.float32)
    w1p[0:C, 0:INTER] = W1.T
    w1p[C:C2, INTER:32] = W1.T
    w2p = np.zeros((32, C2), np.float32)
    w2p[0:INTER, 0:C] = W2.T
    w2p[INTER:32, C:C2] = W2.T
    gw1f = np.concatenate([G1.T, G1.T], axis=0).astype(np.float32)  # [128,16]
    gw2p = np.concatenate([G2.T, G2.T], axis=1)                     # [16,128]

    cw6 = np.zeros((C2, 6, C2), np.float32)
    for jj, (dlt, dx) in enumerate(TAPS):
        for s in (0, 1):
            for p in (0, 1):
                ky = dlt + s + 1 - p
                if 0 <= ky <= 2:
                    cw6[C * s:C * s + C, jj, C * p:C * p + C] = \
                        CW[:, :, ky, dx + 1].T
    # s=1 rows of the d=-1 taps, re-homed to partitions 0:64
    cw3b = np.zeros((C2, 3, C2), np.float32)
    for jj in range(3):
        cw3b[0:C, jj, :] = cw6[C:C2, jj, :]

    wsm = np.zeros((C2, WCOLS), np.float32)
    wsm[:, O_W1P:O_W1P + 32] = w1p
    wsm[0:32, O_W2P:O_W2P + C2] = w2p
    wsm[0:INTER, O_GW2P:O_GW2P + C2] = gw2p
    wsm[0:32, O_BIAS + 0] = np.concatenate([B1, B1])
    wsm[:, O_BIAS + 1] = np.concatenate([B2 + Bg2, B2 + Bg2])
    wsm[:, O_BIAS + 2] = np.concatenate([CB, CB])
    wsm[0:INTER, O_BIAS + 3] = Bg1
    wsm16 = wsm.astype(BFNP)
    # G1 kept as f32 bit-pairs for the f32r mean matmul.  Rounded to bf16
    # mantissa so the low half-words are +0.0 (valid bf16, no fake NaNs).
    gw1f = np.ascontiguousarray(gw1f)
    gw1f = (gw1f.view(np.uint32) & np.uint32(0xFFFF0000)).view(np.float32)
    wsm16[:, O_GW1F:O_GW1F + 32] = gw1f.view(BFNP)

    cwblob = np.zeros((C2, CW_COLS), np.float32)
    cwblob[:, 0:768] = cw6.reshape(C2, 768)
    cwblob[:, 768:1152] = cw3b.reshape(C2, 384)
    return {
        "wsm": wsm16,
        "cw6": cwblob.astype(BFNP),
    }


def make_core_inputs(inputs):
    shared = prepare_weights(inputs)
    x = np.asarray(inputs["x"], dtype=np.float32)
    maps = []
    for i in range(B):
        xi = x[i]                                   # [64, 64, 64]
        xh = np.concatenate([xi[:, 0:H // 2, :].reshape(C, N // 2),
                             xi[:, H // 2:H, :].reshape(C, N // 2)], axis=0)
        xsh = np.concatenate([xi[:, 1:, :],
                              np.zeros((C, 1, W), np.float32)], axis=1)
        xd = np.concatenate([xi.reshape(C, N), xsh.reshape(C, N)], axis=0)
        maps.append({
            "wsm": shared["wsm"],
            "cw6": shared["cw6"],
            "xh": np.ascontiguousarray(xh).astype(BFNP),
            "xd": np.ascontiguousarray(xd).astype(BFNP),
        })
    return maps


def _unpack_y(y2):
    # y2 [128, 2048] bf16: [o, 256j + 64t + w] = y[o, 8j+2t, w];
    # partitions 64:128 hold the odd rows.
    y2 = np.asarray(y2, dtype=np.float32).reshape(2, C, 8, 4, W)
    out = np.empty((C, 8, 4, 2, W), np.float32)
    out[:, :, :, 0, :] = y2[0]
    out[:, :, :, 1, :] = y2[1]
    return out.reshape(C, H, W)


def _run(inputs, trace=False):
    in_maps = make_core_inputs(inputs)
    if "prog" not in _prog_cache:
        _prog_cache["prog"] = build_program(B)
    nc = _prog_cache["prog"]
    res = run_bass_kernel_spmd(nc, in_maps, list(range(B)), trace=trace)
    out = np.stack([_unpack_y(r["y"]) for r in res.results])
    return out.astype(np.float32), res


def kernel(**inputs):
    out, _ = _run(inputs, trace=False)
    return out


def kernel_traced(inputs):
    return _run(inputs, trace=True)


def reference_numpy(inputs):
    """Pure-numpy emulation of the (dead-code-eliminated) reference using the
    same folded weights (f32, no bf16 rounding). Algebra validation only."""
    f = lambda k: np.asarray(inputs[k], dtype=np.float32)
    a1, c1 = _affine(f("ls1"), f("lbb1"), f("lm1"), f("lv1"))
    W1 = a1[:, None] * f("lw1")
    B1 = a1 * f("lb1") + c1
    a2, c2 = _affine(f("ls2"), f("lbb2"), f("lm2"), f("lv2"))
    W2 = a2[:, None] * f("lw2")
    B2 = a2 * f("lb2") + c2
    ag1, cg1 = _affine(f("gs1"), f("gbb1"), f("gm1"), f("gv1"))
    G1 = ag1[:, None] * f("gw1")
    Bg1 = ag1 * f("gb1") + cg1
    ag2, cg2 = _affine(f("gs2"), f("gbb2"), f("gm2"), f("gv2"))
    G2 = ag2[:, None] * f("gw2")
    Bg2 = ag2 * f("gb2") + cg2
    ac, cc = _affine(f("cs"), f("cbb"), f("cm"), f("cv"))
    CW = ac[:, None, None, None] * f("cw")
    CB = ac * f("cb") + cc
    x = np.asarray(inputs["x"], dtype=np.float32)
    out = np.empty_like(x)
    for i in range(B):
        xs = x[i].reshape(C, N)
        t1 = np.maximum(W1 @ xs + B1[:, None], 0.0)
        g = xs.mean(axis=1, keepdims=True)
        d = G2 @ np.maximum(G1 @ g + Bg1[:, None], 0.0) + (B2 + Bg2)[:, None]
        xo = xs / (1.0 + np.exp(-(W2 @ t1 + d)))
        xop = np.zeros((C, H + 2, W + 2), np.float32)
        xop[:, 1:-1, 1:-1] = xo.reshape(C, H, W)
        y = np.zeros((C, N), np.float32)
        for kk in range(9):
            ky, kx = divmod(kk, 3)
            sh = xop[:, ky:ky + H, kx:kx + W].reshape(C, N)
            y += CW[:, :, ky, kx] @ sh
        out[i] = np.maximum(y + CB[:, None], 0.0).reshape(C, H, W)
    return out


# revision 21
# speedup vs baseline: 1.0123x; 1.0123x over previous
"""Trainium2 Bass kernel for nn_Chan_spaAtt (SE-gated conv block), v3.

The spatial self-attention branch in the reference is dead code -- the output
depends only on xo = x * sigmoid(xl + xg) through the final 3x3 conv + BN +
ReLU (all BN affines folded host-side):

  t1   = relu(W1 @ x + b1)                      [16, N]
  d    = G2 @ relu(G1 @ mean(x) + bg1) + bsig   [64, 1]
  sarg = W2 @ t1                                [64, N]
  xo   = x * sigmoid(sarg + d)                  [64, N]
  y    = relu(conv3x3(xo, CW) + cb)             [64, N]

Sharding: one sample per NeuronCore (B=8).

Layout: everything bf16 on-chip, 128 partitions everywhere.
 - x_dual [128, 4096]: partition c+64s holds x[c, row+s] per 8-row chunk.
   The SE phase computes each pixel twice (once per shift) at zero extra
   cost: engine time scales with the free dim only.
 - xo_pad [128, 40*132]: copy A (partitions 0:64) = padded xo grid with
   row stride 66; copy B (64:128) holds the next row's values at the same
   column (written directly by the dual-layout SE multiply).
 - conv3x3 = 6 dense K=128 matmuls per 8-row half-tile: M=128 packs (out
   channel x output-row-parity), K=128 packs (in channel x row shift).
   12288 PE rows total vs 24576 in the 9-tap formulation.  The first
   half-tile replaces its three K=128 taps with K=64 pairs so it never
   reads copy B's unwritten row -1 column (no fixup DMA on the chain).
 - global-branch mean via DVE reduce over a [128, 2048] half-stacked copy
   of x; a stacked-G1 f32r matmul recombines the partition halves exactly.
 - DMA: HWDGE descriptor-gen costs a flat ~625ns serialized per transfer,
   so x_dual rides the Pool-engine SWDGE path and everything else is
   batched into few transfers, ordered so the mean-reduce stream lands
   first.
"""

import sys

if "/opt/trn_rl_repo" not in sys.path:
    sys.path.insert(0, "/opt/trn_rl_repo")

import numpy as np
import ml_dtypes

import concourse.bass as bass
import concourse.bacc as bacc
import concourse.mybir as mybir
import concourse.tile as tile
from concourse.bass_utils import run_bass_kernel_spmd

B, C, H, W = 8, 64, 64, 64
N = H * W
C2 = 2 * C          # 128
INTER = 16
EPS = 1e-5
PW = W + 2          # padded row stride = 66
BW = 2 * PW         # conv-view block width = 132 (one row pair)
NBLK = 40           # blocks in xo_pad; 40*132 = 5280 columns
PADC = NBLK * BW
HEAD = PW + 1       # flat offset of grid pixel (0, 0) = 67
CHUNK = 512
NCHUNK = N // CHUNK          # 8
HALF = 256                   # conv half-tile free size (4 row pairs)

TAPS = ((-1, -1), (-1, 0), (-1, 1), (1, -1), (1, 0), (1, 1))

F32 = mybir.dt.float32
F32R = mybir.dt.float32r
BF16 = mybir.dt.bfloat16
AF = mybir.ActivationFunctionType
ALU = mybir.AluOpType
BFNP = ml_dtypes.bfloat16

# weight blob (bf16, 128 partitions) column layout
O_W1P = 0      # [128, 32]
O_W2P = 32     # [32, 128] on partitions 0:32
O_GW1F = 160   # [128, 64] = [128, 32] f32 (bitcast), stacked G1
O_GW2P = 224   # [16, 128] on partitions 0:16
O_BIAS = 352   # 4 f32-as-2xbf16? no: 4 bf16 cols: b1 | bsig | cb | gb1
WCOLS = 356
# cw6 blob: 6 dense taps [128, 768] + cw3b (s=1 rows of the d=-1 taps,
# re-homed to partitions 0:64) [64, 384] at cols 768:1152
CW_COLS = 1152

XH_SPLITS = ((0, 1024), (1024, 2048))
XD_SPLITS = ((0, 1024), (1024, 2048), (2048, 4096))

_prog_cache = {}


def _pix(r, w):
    """Flat column of valid grid pixel (r, w) in xo_pad copy A."""
    return HEAD + r * PW + w


def build_program(n_cores=8):
    nc = bacc.Bacc("TRN2", debug=False, target_bir_lowering=False,
                   num_devices=n_cores)

    wsm_d = nc.dram_tensor("wsm", [C2, WCOLS], BF16, kind="ExternalInput").ap()
    xh_d = nc.dram_tensor("xh", [C2, N // 2], BF16, kind="ExternalInput").ap()
    xd_d = nc.dram_tensor("xd", [C2, N], BF16, kind="ExternalInput").ap()
    cw6_d = nc.dram_tensor("cw6", [C2, CW_COLS], BF16,
                           kind="ExternalInput").ap()
    y_d = nc.dram_tensor("y", [C2, N // 2], BF16, kind="ExternalOutput").ap()

    with tile.TileContext(nc) as tc:
        with tc.tile_pool(name="big", bufs=1) as bpool, \
             tc.tile_pool(name="work", bufs=3) as wpool, \
             tc.tile_pool(name="t1s", bufs=8) as tpool, \
             tc.tile_pool(name="ps1p", bufs=2, space="PSUM") as pp1, \
             tc.tile_pool(name="ps2p", bufs=3, space="PSUM") as pp2, \
             tc.tile_pool(name="psyp", bufs=3, space="PSUM") as ppy:

            # dummy sigmoid at t~0: forces the single needed ACT table set
            # (sigmoid_and_others: sigmoid + relu + identity) to load early.
            scr = bpool.tile([1, 1], F32, tag="scr")
            nc.vector.memset(scr[:], 0)
            nc.scalar.activation(scr[:], scr[:], AF.Sigmoid)

            # ---- input DMAs.  SP/HWDGE: mean-reduce stream, weights.
            # Pool/SWDGE: x_dual pieces, delayed behind the halo memsets so
            # the xh stream wins the DMA-engine arbitration. ----
            xh = bpool.tile([C2, N // 2], BF16, tag="xh")
            for lo, hi in XH_SPLITS:
                nc.sync.dma_start(xh[:, lo:hi], xh_d[:, lo:hi])
            wsm = bpool.tile([C2, WCOLS], BF16, tag="wsm")
            nc.sync.dma_start(wsm[:], wsm_d)
            cw6 = bpool.tile([C2, CW_COLS], BF16, tag="cw6")
            nc.sync.dma_start(cw6[:], cw6_d)

            # ---- xo_pad halo memsets on Pool (they also pace the SWDGE
            # x_dual gens behind the xh transfers) ----
            xo_pad = bpool.tile([C2, PADC], BF16, tag="xopad")
            nc.gpsimd.memset(xo_pad[:, 0:HEAD], 0)
            gaps = xo_pad[:, HEAD + W:HEAD + W + H * PW]
            gaps = gaps.rearrange("p (r w) -> p r w", w=PW)[:, :, 0:2]
            nc.gpsimd.memset(gaps, 0)
            # cover reads up to conv block 32 (col 4356); beyond is never read
            nc.gpsimd.memset(xo_pad[:, _pix(H - 1, W) + 2:33 * BW], 0)
            # copy B's slot for grid row 64 (the bottom halo) stays zero
            nc.gpsimd.memset(xo_pad[C:C2, _pix(H - 1, 0):_pix(H - 1, W)], 0)

            xd = bpool.tile([C2, N], BF16, tag="xd")
            for lo, hi in XD_SPLITS:
                nc.gpsimd.dma_start(xd[:, lo:hi], xd_d[:, lo:hi])

            w1p = wsm[:, O_W1P:O_W1P + 32]
            w2p = wsm[0:32, O_W2P:O_W2P + C2]
            gw1f = wsm[:, O_GW1F:O_GW1F + 32].bitcast(F32R)
            gw2p = wsm[0:INTER, O_GW2P:O_GW2P + C2]

            # ---- global mean partials: one DVE reduce per xh piece ----
            gparts = wpool.tile([C2, 2], F32, tag="gparts")
            for q in range(2):
                lo, hi = XH_SPLITS[q]
                nc.vector.reduce_sum(gparts[:, q:q + 1], xh[:, lo:hi],
                                     axis=mybir.AxisListType.X)

            # ---- f32 bias columns, converted on-chip from the bf16 blob ----
            fbias = wpool.tile([C2, 4], F32, tag="fbias")
            nc.vector.tensor_copy(fbias[:], wsm[:, O_BIAS:O_BIAS + 4])
            b1 = fbias[0:32, 0:1]
            bsig = fbias[:, 1:2]
            cb = fbias[:, 2:3]
            gb1 = fbias[0:INTER, 3:4]

            # ---- SE phase 1: mm1 for every chunk; t1 relu spread over
            # ACT (c0), Pool (odd), DVE (even, emitted inside phase 2) ----
            t1s = {}
            ps1s = {}

            def emit_mm1(ci):
                ps1 = pp1.tile([32, CHUNK], F32, tag="ps1")
                nc.tensor.matmul(ps1[:], w1p,
                                 xd[:, ci * CHUNK:(ci + 1) * CHUNK],
                                 start=True, stop=True)
                ps1s[ci] = ps1

            def emit_t1(ci, eng):
                t1 = tpool.tile([32, CHUNK], BF16, tag="t1")
                if eng == "act":
                    nc.scalar.activation(t1[:], ps1s[ci][:], AF.Relu, bias=b1)
                elif eng == "pool":
                    nc.gpsimd.tensor_scalar(t1[:], ps1s[ci][:], b1, 0.0,
                                            ALU.add, ALU.max)
                else:
                    nc.vector.tensor_scalar(t1[:], ps1s[ci][:], b1, 0.0,
                                            ALU.add, ALU.max)
                t1s[ci] = t1

            emit_mm1(0)
            emit_t1(0, "act")
            emit_mm1(1)
            emit_t1(1, "pool")

            # ---- global branch MLP; gmm1 accumulates the two mean
            # partials directly (no combine reduce) ----
            psg1 = ppy.tile([INTER, 1], F32, tag="psy")
            nc.tensor.matmul(psg1[:], gw1f, gparts[:, 0:1].bitcast(F32R),
                             start=True, stop=False)
            nc.tensor.matmul(psg1[:], gw1f, gparts[:, 1:2].bitcast(F32R),
                             start=False, stop=True)
            g1 = wpool.tile([INTER, 1], BF16, tag="g1")
            nc.scalar.activation(g1[:], psg1[:], AF.Relu, bias=gb1,
                                 scale=1.0 / N)
            psg2 = pp2.tile([C2, 1], F32, tag="ps2")
            nc.tensor.matmul(psg2[:], gw2p, g1[:], start=True, stop=True)
            dbias = wpool.tile([C2, 1], F32, tag="dbias")
            nc.scalar.activation(dbias[:], psg2[:], AF.Identity, bias=bsig)

            # ---- SE phase 2 + conv, software-pipelined ----
            def emit_mm2_sig(ci):
                ps2 = pp2.tile([C2, CHUNK], F32, tag="ps2")
                nc.tensor.matmul(ps2[:], w2p, t1s[ci][:],
                                 start=True, stop=True)
                sig = wpool.tile([C2, CHUNK], BF16, tag="sig")
                nc.scalar.activation(sig[:], ps2[:], AF.Sigmoid,
                                     bias=dbias[:])
                return sig

            def mul_rows(ci, sig, r0, nrow, top_only=False):
                pbase = C if top_only else C2
                off = (r0 - 8 * ci) * W
                dst = xo_pad[0:pbase, _pix(r0, 0):_pix(r0, 0) + nrow * PW]
                dst = dst.rearrange("p (r w) -> p r w", w=PW)[:, :, 0:W]
                xcr = xd[0:pbase, ci * CHUNK + off:ci * CHUNK + off + nrow * W]
                xcr = xcr.rearrange("p (r w) -> p r w", w=W)
                sgr = sig[0:pbase, off:off + nrow * W]
                sgr = sgr.rearrange("p (r w) -> p r w", w=W)
                nc.vector.tensor_mul(dst, xcr, sgr)

            def emit_mul(ci, sig):
                if ci < NCHUNK - 1:
                    mul_rows(ci, sig, 8 * ci, 8)
                else:
                    # split so conv h6 (needs only row 56) unblocks early;
                    # the bottom half's value for row 64 is never written
                    # (copy B's slot for it is the zero bottom halo)
                    mul_rows(ci, sig, 56, 1)
                    mul_rows(ci, sig, 57, 6)
                    mul_rows(ci, sig, 63, 1, top_only=True)

            # ---- conv3x3: 6 dense 128x128 matmuls per 8-row half-tile;
            # half 0 splits its d=-1 taps into K=64 pairs (no copy-B read
            # of the unwritten row -1 column). ----
            xor_v = xo_pad[:].rearrange("p (t w) -> p t w", w=BW)
            # shifted top-half view whose row-pair t holds content row 2t
            # (used by half 0 in place of copy B)
            xob_v = xo_pad[0:C, PW:PW + 4 * BW]
            xob_v = xob_v.rearrange("p (t w) -> p t w", w=BW)
            ysb = bpool.tile([C2, N // 2], BF16, tag="ysb")

            def emit_conv_half(j):
                psy = ppy.tile([C2, HALF], F32, tag="psy")
                first = True
                for dlt, dx in ((1, -1), (1, 0), (1, 1),
                                (-1, -1), (-1, 0), (-1, 1)):
                    jj = TAPS.index((dlt, dx))
                    wcol = jj * C2
                    t0 = (8 * j + dlt + 1) // 2
                    if j == 0 and dlt == -1:
                        nc.tensor.matmul(
                            psy[:], cw6[0:C, wcol:wcol + C2],
                            xor_v[0:C, t0:t0 + 4, 1 + dx:1 + dx + W],
                            start=False, stop=False)
                        kk = 768 + jj * C2
                        nc.tensor.matmul(
                            psy[:], cw6[0:C, kk:kk + C2],
                            xob_v[:, 0:4, 1 + dx:1 + dx + W],
                            start=False, stop=(dx == 1))
                    else:
                        nc.tensor.matmul(
                            psy[:], cw6[:, wcol:wcol + C2],
                            xor_v[:, t0:t0 + 4, 1 + dx:1 + dx + W],
                            start=first, stop=(j > 0 and dlt == -1
                                               and dx == 1))
                    first = False
                dsty = ysb[:, j * HALF:(j + 1) * HALF]
                if j < 4:
                    nc.gpsimd.tensor_scalar(dsty, psy[:], cb, 0.0,
                                            ALU.add, ALU.max)
                else:
                    nc.scalar.activation(dsty, psy[:], AF.Relu, bias=cb)

            sig0 = emit_mm2_sig(0)
            emit_mul(0, sig0)
            emit_mm1(2)
            emit_t1(2, "dve")
            sig1 = emit_mm2_sig(1)
            emit_mul(1, sig1)
            emit_mm1(3)
            emit_t1(3, "pool")
            sig2 = emit_mm2_sig(2)
            emit_mul(2, sig2)
            emit_mm1(4)
            emit_t1(4, "dve")
            emit_conv_half(0)
            sig3 = emit_mm2_sig(3)
            emit_mul(3, sig3)
            emit_mm1(5)
            emit_t1(5, "pool")
            emit_conv_half(1)
            sig4 = emit_mm2_sig(4)
            emit_mul(4, sig4)
            emit_mm1(6)
            emit_t1(6, "dve")
            emit_conv_half(2)
            sig5 = emit_mm2_sig(5)
            emit_mul(5, sig5)
            emit_mm1(7)
            emit_t1(7, "pool")
            emit_conv_half(3)
            sig6 = emit_mm2_sig(6)
            emit_mul(6, sig6)
            emit_conv_half(4)
            sig7 = emit_mm2_sig(7)
            emit_mul(7, sig7)
            emit_conv_half(5)
            nc.sync.dma_start(y_d[:, 0:4 * HALF], ysb[:, 0:4 * HALF])
            emit_conv_half(6)
            emit_conv_half(7)
            nc.sync.dma_start(y_d[:, 4 * HALF:7 * HALF],
                              ysb[:, 4 * HALF:7 * HALF])
            nc.sync.dma_start(y_d[:, 7 * HALF:8 * HALF],
                              ysb[:, 7 * HALF:8 * HALF])

    nc.compile()
    return nc


def _affine(s, b, m, v):
    inv = s / np.sqrt(v + EPS)
    return inv, b - m * inv


def prepare_weights(inputs):
    f = lambda k: np.asarray(inputs[k], dtype=np.float32)
    a1, c1 = _affine(f("ls1"), f("lbb1"), f("lm1"), f("lv1"))
    W1 = a1[:, None] * f("lw1")              # [16, 64]
    B1 = a1 * f("lb1") + c1
    a2, c2 = _affine(f("ls2"), f("lbb2"), f("lm2"), f("lv2"))
    W2 = a2[:, None] * f("lw2")              # [64, 16]
    B2 = a2 * f("lb2") + c2
    ag1, cg1 = _affine(f("gs1"), f("gbb1"), f("gm1"), f("gv1"))
    G1 = ag1[:, None] * f("gw1")             # [16, 64]
    Bg1 = ag1 * f("gb1") + cg1
    ag2, cg2 = _affine(f("gs2"), f("gbb2"), f("gm2"), f("gv2"))
    G2 = ag2[:, None] * f("gw2")             # [64, 16]
    Bg2 = ag2 * f("gb2") + cg2
    ac, cc = _affine(f("cs"), f("cbb"), f("cm"), f("cv"))
    CW = ac[:, None, None, None] * f("cw")   # [64, 64, 3, 3] (o, c, ky, kx)
    CB = ac * f("cb") + cc

    w1p = np.zeros((C2, 32), np.float32)
    w1p[0:C, 0:INTER] = W1.T
    w1p[C:C2, INTER:32] = W1.T
    w2p = np.zeros((32, C2), np.float32)
    w2p[0:INTER, 0:C] = W2.T
    w2p[INTER:32, C:C2] = W2.T
    gw1f = np.concatenate([G1.T, G1.T], axis=0).astype(np.float32)  # [128,16]
    gw2p = np.concatenate([G2.T, G2.T], axis=1)                     # [16,128]

    cw6 = np.zeros((C2, 6, C2), np.float32)
    for jj, (dlt, dx) in enumerate(TAPS):
        for s in (0, 1):
            for p in (0, 1):
                ky = dlt + s + 1 - p
                if 0 <= ky <= 2:
                    cw6[C * s:C * s + C, jj, C * p:C * p + C] = \
                        CW[:, :, ky, dx + 1].T
    # s=1 rows of the d=-1 taps, re-homed to partitions 0:64
    cw3b = np.zeros((C2, 3, C2), np.float32)
    for jj in range(3):
        cw3b[0:C, jj, :] = cw6[C:C2, jj, :]

    wsm = np.zeros((C2, WCOLS), np.float32)
    wsm[:, O_W1P:O_W1P + 32] = w1p
    wsm[0:32, O_W2P:O_W2P + C2] = w2p
    wsm[0:INTER, O_GW2P:O_GW2P + C2] = gw2p
    wsm[0:32, O_BIAS + 0] = np.concatenate([B1, B1])
    wsm[:, O_BIAS + 1] = np.concatenate([B2 + Bg2, B2 + Bg2])
    wsm[:, O_BIAS + 2] = np.concatenate([CB, CB])
    wsm[0:INTER, O_BIAS + 3] = Bg1
    wsm16 = wsm.astype(BFNP)
    # G1 kept as f32 bit-pairs for the f32r mean matmul.  Rounded to bf16
    # mantissa so the low half-words are +0.0 (valid bf16, no fake NaNs).
    gw1f = np.ascontiguousarray(gw1f)
    gw1f = (gw1f.view(np.uint32) & np.uint32(0xFFFF0000)).view(np.float32)
    wsm16[:, O_GW1F:O_GW1F + 32] = gw1f.view(BFNP)

    cwblob = np.zeros((C2, CW_COLS), np.float32)
    cwblob[:, 0:768] = cw6.reshape(C2, 768)
    cwblob[:, 768:1152] = cw3b.reshape(C2, 384)
    return {
        "wsm": wsm16,
        "cw6": cwblob.astype(BFNP),
    }


def make_core_inputs(inputs):
    shared = prepare_weights(inputs)
    x = np.asarray(inputs["x"], dtype=np.float32)
    maps = []
    for i in range(B):
        xi = x[i]                                   # [64, 64, 64]
        xh = np.concatenate([xi[:, 0:H // 2, :].reshape(C, N // 2),
                             xi[:, H // 2:H, :].reshape(C, N // 2)], axis=0)
        xsh = np.concatenate([xi[:, 1:, :],
                              np.zeros((C, 1, W), np.float32)], axis=1)
        xd = np.concatenate([xi.reshape(C, N), xsh.reshape(C, N)], axis=0)
        maps.append({
            "wsm": shared["wsm"],
            "cw6": shared["cw6"],
            "xh": np.ascontiguousarray(xh).astype(BFNP),
            "xd": np.ascontiguousarray(xd).astype(BFNP),
        })
    return maps


def _unpack_y(y2):
    # y2 [128, 2048] bf16: [o, 256j + 64t + w] = y[o, 8j+2t, w];
    # partitions 64:128 hold the odd rows.
    y2 = np.asarray(y2, dtype=np.float32).reshape(2, C, 8, 4, W)
    out = np.empty((C, 8, 4, 2, W), np.float32)
    out[:, :, :, 0, :] = y2[0]
    out[:, :, :, 1, :] = y2[1]
    return out.reshape(C, H, W)


def _run(inputs, trace=False):
    in_maps = make_core_inputs(inputs)
    if "prog" not in _prog_cache:
        _prog_cache["prog"] = build_program(B)
    nc = _prog_cache["prog"]
    res = run_bass_kernel_spmd(nc, in_maps, list(range(B)), trace=trace)
    out = np.stack([_unpack_y(r["y"]) for r in res.results])
    return out.astype(np.float32), res


def kernel(**inputs):
    out, _ = _run(inputs, trace=False)
    return out


def kernel_traced(inputs):
    return _run(inputs, trace=True)


def reference_numpy(inputs):
    """Pure-numpy emulation of the (dead-code-eliminated) reference using the
    same folded weights (f32, no bf16 rounding). Algebra validation only."""
    f = lambda k: np.asarray(inputs[k], dtype=np.float32)
    a1, c1 = _affine(f("ls1"), f("lbb1"), f("lm1"), f("lv1"))
    W1 = a1[:, None] * f("lw1")
    B1 = a1 * f("lb1") + c1
    a2, c2 = _affine(f("ls2"), f("lbb2"), f("lm2"), f("lv2"))
    W2 = a2[:, None] * f("lw2")
    B2 = a2 * f("lb2") + c2
    ag1, cg1 = _affine(f("gs1"), f("gbb1"), f("gm1"), f("gv1"))
    G1 = ag1[:, None] * f("gw1")
    Bg1 = ag1 * f("gb1") + cg1
    ag2, cg2 = _affine(f("gs2"), f("gbb2"), f("gm2"), f("gv2"))
    G2 = ag2[:, None] * f("gw2")
    Bg2 = ag2 * f("gb2") + cg2
    ac, cc = _affine(f("cs"), f("cbb"), f("cm"), f("cv"))
    CW = ac[:, None, None, None] * f("cw")
    CB = ac * f("cb") + cc
    x = np.asarray(inputs["x"], dtype=np.float32)
    out = np.empty_like(x)
    for i in range(B):
        xs = x[i].reshape(C, N)
        t1 = np.maximum(W1 @ xs + B1[:, None], 0.0)
        g = xs.mean(axis=1, keepdims=True)
        d = G2 @ np.maximum(G1 @ g + Bg1[:, None], 0.0) + (B2 + Bg2)[:, None]
        xo = xs / (1.0 + np.exp(-(W2 @ t1 + d)))
        xop = np.zeros((C, H + 2, W + 2), np.float32)
        xop[:, 1:-1, 1:-1] = xo.reshape(C, H, W)
        y = np.zeros((C, N), np.float32)
        for kk in range(9):
            ky, kx = divmod(kk, 3)
            sh = xop[:, ky:ky + H, kx:kx + W].reshape(C, N)
            y += CW[:, :, ky, kx] @ sh
        out[i] = np.maximum(y + CB[:, None], 0.0).reshape(C, H, W)
    return out
